# revision 30
# baseline (speedup 1.0000x reference)
"""Boundary-distance loss (BDLoss) on 8 Trainium2 NeuronCores.

Math (matches the reference):
  probs = softmax(net_output, axis=1)
  onehot_c = (gt == c)
  posdis = EDT(onehot_c), negdis = EDT(~onehot_c)
  phi = where(inner_boundary, 0, negdis - posdis), zeroed if class absent
  out  = mean(probs[:, 1:] * phi[:, 1:])

Algorithm (exponential-weight separable convolution):
  * NEG field: E = conv3d(fpos, beta^-d^2) over the 5^3 box with beta = 2^8.
    Since all weights are powers of two and counts per distance-shell are
    < 32, the EXPONENT FIELD of E encodes m = min d^2 exactly:
    x := (bits(E) + 512) >> 10 == 16 - m (x == 0 for "nothing in box").
    The conv is separable: the x-taps ride in banded 128x128 matmul weights,
    the y-taps are 5 shifted matmul passes accumulated in PSUM (all on PE),
    and the z-taps are 4 pair-adds + 2 scales on DVE.  sqrt(m) is a single
    ACT pass: sqrt(-1*x + 16).
  * POS field: posE = conv3d(fneg, beta^-d^2) over the 3^3 box (fneg = "bg
    for class c", with OOV/pads forced to 0 so out-of-volume never counts
    as background).  Then the boundary-zeroed positive distance is
    posd = sqrt2*[posE < 2^-9] + (sqrt3-sqrt2)*[posE < 2^-17]
    (thresholds = "no bg at d^2<=1 / <=2"), auto-zero on bg voxels because
    the center tap makes posE >= 1 there.
  * phi = sqrt(m) - posd; result = sum(probs_c * phi) via a DVE
    tensor_scalar with accum_out per class; host sums across cores.
  * Verification: x-fields are shipped; min(x) >= 8 iff the windowed box
    EDT is exact (neg^2 <= 8 everywhere); pos exactness (pos^2 <= 3) is
    checked with numpy erosion.  On failure -> exact scipy fallback.

Sharding: core = (b, z-slab): b = core//4, z0 = 24*(core%4).  gt is sent as
uint16 with a 2-plane z halo and 2-column y pad of value 255.
"""

import numpy as np
import ml_dtypes

import concourse.bacc as bacc
import concourse.mybir as mybir
from concourse.tile import TileContext
from concourse import bass_utils

F32 = mybir.dt.float32
BF16 = mybir.dt.bfloat16
U16 = mybir.dt.uint16
AL = mybir.AluOpType
AF = mybir.ActivationFunctionType

B, C, X, Y, Z = 2, 4, 128, 128, 96
ZO = 24            # output z-planes per core
H = 2              # z halo
ZT = ZO + 2 * H    # 28 z-planes on chip
YP = Y + 4         # y padded to 132
FDH = YP * ZT      # 3696 cols of the padded mask tile
FDO = Y * ZO       # 3072 cols of a dense output tile
NVOX = B * (C - 1) * X * Y * Z
SQ2 = float(np.sqrt(2.0))
SQ3 = float(np.sqrt(3.0))
LB = 2.0 ** -8     # conv base beta^-1
T1 = 2.0 ** -9     # pos threshold: no bg at d^2 <= 1
T2 = 2.0 ** -17    # pos threshold: no bg at d^2 <= 2
CW = 3584          # conv output cols: y in [2,130) x z in [0,28)
C_LO = 2 * ZT      # first output col
CHUNK = 1024       # psum chunk (2 banks)
SUB = 512          # matmul sub-block (1 bank)

NEG_PASSES = [(2, -2), (2, 2), (1, -1), (1, 1), (0, 0)]   # (wn band idx, dy)
POS_PASSES = [(1, -1), (1, 1), (0, 0)]                    # (wp band idx, dy)


def _body(tc, gt_d, net_d, wn_d, wp_d, out_d, x_d):
    nc = tc.nc
    with tc.tile_pool(name="main", bufs=1) as pool, \
         tc.tile_pool(name="rot", bufs=2) as rot, \
         tc.tile_pool(name="nps", bufs=2, space="PSUM") as nps, \
         tc.tile_pool(name="pps", bufs=2, space="PSUM") as pps:

        # ---- input loads --------------------------------------------
        gt16 = pool.tile([128, FDH], U16, tag="gt")
        for gg in range(2):
            sl = slice(gg * FDH // 2, (gg + 1) * FDH // 2)
            nc.sync.dma_start(gt16[:, sl], gt_d[:, sl])
        wn_t = pool.tile([128, 384], BF16, tag="wn")
        nc.sync.dma_start(wn_t[:, :], wn_d)
        wp_t = pool.tile([128, 512], BF16, tag="wp")
        nc.sync.dma_start(wp_t[:, :], wp_d)
        b16 = pool.tile([128, 1], F32, tag="b16")
        nc.vector.memset(b16[:, :], 16.0)
        junk = pool.tile([128, 384], BF16, tag="junk")
        nc.vector.memset(junk[:, :], 0.0)

        # ---- softmax exps: 2 now, 6 after class-1 copy emission ------
        e_t = pool.tile([128, 4 * FDO], BF16, tag="et")
        HF = FDO // 2

        def emit_exps(rng):
            for cc in rng:
                st = rot.tile([128, HF], F32, tag="stage", bufs=3)
                nc.sync.dma_start(st[:, :], net_d[:, cc * HF:(cc + 1) * HF])
                nc.scalar.activation(e_t[:, cc * HF:(cc + 1) * HF], st[:, :],
                                     AF.Exp)

        emit_exps(range(2))

        # ---- PE warm-up: keep the tensor engine busy from t~0.6us so
        # the p-state model ramps to full speed before the real convs
        # (a stalled wait resets the ramp; these chain into chunk 0's
        # psum, which the first real pass resets via start=True).
        warm = nps.tile([128, CHUNK], F32, tag="exyps", bufs=2)
        for _ in range(13):
            nc.tensor.matmul(warm[:, 0:384], junk[:, 0:128], junk[:, :],
                             start=True, stop=True)

        gtv = gt16[:, :].rearrange("p (y z) -> p y z", z=ZT)

        def build_masks(c, nch=1):
            fpos = rot.tile([128, FDH], BF16, tag="fpos", bufs=3)
            fneg = rot.tile([128, FDH], BF16, tag="fneg", bufs=3)
            for g in range(nch):
                sl = slice(g * FDH // nch, (g + 1) * FDH // nch)
                nc.vector.tensor_scalar(fpos[:, sl], gt16[:, sl], c, None,
                                        AL.is_equal)
            for g in range(nch):
                sl = slice(g * FDH // nch, (g + 1) * FDH // nch)
                nc.vector.tensor_scalar(fneg[:, sl], gt16[:, sl], c, None,
                                        AL.not_equal)
            # y pads -> 0 (OOV is never background)
            nc.gpsimd.memset(fneg[:, 0:2 * ZT], 0.0)
            nc.gpsimd.memset(fneg[:, 130 * ZT:132 * ZT], 0.0)
            return fpos, fneg

        def halo_fix(fpos, fneg):
            # z halo -> V - fpos (0 on OOV pad, unchanged on real data)
            fnv = fneg[:, :].rearrange("p (y z) -> p y z", z=ZT)
            fpv = fpos[:, :].rearrange("p (y z) -> p y z", z=ZT)
            nc.vector.tensor_tensor(fnv[:, :, 0:2], vhv[:, :, 0:2],
                                    fpv[:, :, 0:2], AL.subtract)
            nc.vector.tensor_tensor(fnv[:, :, ZT - 2:ZT], vhv[:, :, 2:4],
                                    fpv[:, :, ZT - 2:ZT], AL.subtract)

        den = pool.tile([128, FDO], BF16, tag="den")
        out_t = pool.tile([128, 4], F32, tag="out")
        scr = e_t[:, 0:FDO]        # e0 slot is dead once den exists
        xzs, gs = {}, {}
        masks = [build_masks(1, nch=2)]
        # valid-mask z-halo planes (for the fneg halo fix), after masks so
        # the DVE queue serves class-1's masks first
        vh = pool.tile([128, 4 * YP], BF16, tag="vh")
        vhv = vh[:, :].rearrange("p (y z) -> p y z", z=4)
        nc.vector.tensor_scalar(vhv[:, :, 0:2], gtv[:, :, 0:2], 3, None,
                                AL.is_le)
        nc.vector.tensor_scalar(vhv[:, :, 2:4], gtv[:, :, ZT - 2:ZT], 3,
                                None, AL.is_le)
        halo_fix(*masks[0])

        def conv_xy(mask, passes, wmat, tag, ppool):
            """x-band (x) y-shift (xy) conv passes -> SBUF bf16 [128, CW]."""
            sb = rot.tile([128, CW], BF16, tag=tag, bufs=2)
            off = 0
            while off < CW:
                w = min(CHUNK, CW - off)
                ps = ppool.tile([128, CHUNK], F32, tag=tag + "ps", bufs=2)
                for pi, (wi, dy) in enumerate(passes):
                    first, last = pi == 0, pi == len(passes) - 1
                    for mm in range(0, w, SUB):
                        mw = min(SUB, w - mm)
                        a = C_LO + dy * ZT + off + mm
                        nc.tensor.matmul(ps[:, mm:mm + mw],
                                         wmat[:, 128 * wi:128 * (wi + 1)],
                                         mask[:, a:a + mw],
                                         start=first, stop=last)
                nc.scalar.activation(sb[:, off:off + w], ps[:, 0:w], AF.Copy)
                off += w
            return sb

        YH = Y // 2

        def zconv5(sb):
            """5-tap z-conv on DVE (y-halves) -> flat [128, FDO] bf16."""
            v = sb[:, :].rearrange("p (y z) -> p y z", z=ZT)
            a = rot.tile([128, FDO], BF16, tag="A", bufs=2)
            b = rot.tile([128, FDO], BF16, tag="Bt", bufs=2)
            av = a[:, :].rearrange("p (y z) -> p y z", z=ZO)
            bv = b[:, :].rearrange("p (y z) -> p y z", z=ZO)
            for y0 in (0, YH):
                ys = slice(y0, y0 + YH)
                nc.vector.tensor_tensor(av[:, ys, :], v[:, ys, 1:1 + ZO],
                                        v[:, ys, 3:3 + ZO], AL.add)
                nc.vector.tensor_tensor(bv[:, ys, :], v[:, ys, 0:ZO],
                                        v[:, ys, 4:4 + ZO], AL.add)
                nc.vector.tensor_scalar(av[:, ys, :], av[:, ys, :], LB,
                                        None, AL.mult)
                nc.vector.tensor_scalar(bv[:, ys, :], bv[:, ys, :], LB ** 4,
                                        None, AL.mult)
                nc.vector.tensor_tensor(av[:, ys, :], v[:, ys, 2:2 + ZO],
                                        av[:, ys, :], AL.add)
                nc.vector.tensor_tensor(av[:, ys, :], av[:, ys, :],
                                        bv[:, ys, :], AL.add)
            return a, b

        def zconv3(sb):
            """3-tap z-conv on DVE (y-halves) -> flat [128, FDO] bf16."""
            v = sb[:, :].rearrange("p (y z) -> p y z", z=ZT)
            a = rot.tile([128, FDO], BF16, tag="Bt", bufs=2)
            av = a[:, :].rearrange("p (y z) -> p y z", z=ZO)
            for y0 in (0, YH):
                ys = slice(y0, y0 + YH)
                nc.vector.tensor_tensor(av[:, ys, :], v[:, ys, 1:1 + ZO],
                                        v[:, ys, 3:3 + ZO], AL.add)
                nc.vector.tensor_scalar(av[:, ys, :], av[:, ys, :], LB,
                                        None, AL.mult)
                nc.vector.tensor_tensor(av[:, ys, :], v[:, ys, 2:2 + ZO],
                                        av[:, ys, :], AL.add)
            return a

        for ci in range(3):
            fpos, fneg = masks[ci]
            exy = conv_xy(fpos, NEG_PASSES, wn_t, "exy", nps)
            cxy = conv_xy(fneg, POS_PASSES, wp_t, "cxy", pps)
            if ci < 2:
                masks.append(build_masks(ci + 2))
                halo_fix(*masks[ci + 1])
            if ci == 0:
                emit_exps(range(2, 8))
                # den + 1/den + probability folds (Pool + ACT, off-path)
                nc.gpsimd.tensor_tensor(den[:, :], e_t[:, 0:FDO],
                                        e_t[:, FDO:2 * FDO], AL.add)
                nc.gpsimd.tensor_tensor(den[:, :], den[:, :],
                                        e_t[:, 2 * FDO:3 * FDO], AL.add)
                nc.gpsimd.tensor_tensor(den[:, :], den[:, :],
                                        e_t[:, 3 * FDO:4 * FDO], AL.add)
                for hh in range(2):
                    sl = slice(hh * HF, (hh + 1) * HF)
                    lh = rot.tile([128, HF], F32, tag="stage", bufs=3)
                    nc.scalar.activation(lh[:, :], den[:, sl], AF.Ln)
                    nc.scalar.activation(den[:, sl], lh[:, :], AF.Exp,
                                         scale=-1.0)
                for c in (1, 2, 3):
                    esl = e_t[:, c * FDO:(c + 1) * FDO]
                    nc.gpsimd.tensor_tensor(esl, esl, den[:, :], AL.mult)
            # neg z + exponent decode -> x = 16 - m (u16)
            ez, bt = zconv5(exy)
            xz = rot.tile([128, FDO], U16, tag="xz", bufs=3)
            nc.vector.tensor_scalar(xz[:, :], ez[:, :].bitcast(U16), 10,
                                    None, AL.logical_shift_right)
            nc.sync.dma_start(x_d[:, ci * FDO:(ci + 1) * FDO], xz[:, :])
            xzs[ci] = xz
            # pos z + thresholds -> g = posd'
            pe = zconv3(cxy)
            g1 = rot.tile([128, FDO], BF16, tag="g", bufs=3)
            g2 = rot.tile([128, FDO], BF16, tag="Bt", bufs=2)
            nc.vector.tensor_scalar(g1[:, :], pe[:, :], T1, SQ2, AL.is_lt,
                                    AL.mult)
            nc.vector.tensor_scalar(g2[:, :], pe[:, :], T2, SQ3 - SQ2,
                                    AL.is_lt, AL.mult)
            nc.vector.tensor_tensor(g1[:, :], g1[:, :], g2[:, :], AL.add)
            gs[ci] = g1

        # ---- sqrts batched (one act-table switch), then tails --------
        for ci in range(3):
            sn = rot.tile([128, FDO], BF16, tag="sn", bufs=2)
            nc.scalar.activation(sn[:, :], xzs[ci][:, :], AF.Sqrt,
                                 bias=b16[:, 0:1], scale=-1.0)
            g = gs[ci]
            nc.vector.tensor_tensor(g[:, :], sn[:, :], g[:, :],
                                    AL.subtract)
            nc.vector.tensor_tensor(g[:, :], g[:, :],
                                    e_t[:, (ci + 1) * FDO:(ci + 2) * FDO],
                                    AL.mult)
            nc.vector.tensor_scalar(scr[:, :], g[:, :], 1.0, 0.0, AL.mult,
                                    AL.add, accum_out=out_t[:, ci:ci + 1])
        nc.vector.memset(out_t[:, 3:4], 0.0)
        nc.sync.dma_start(out_d[:, :], out_t[:, :])


_NC = None


def _get_nc():
    global _NC
    if _NC is None:
        nc = bacc.Bacc("TRN2", target_bir_lowering=False, debug=False,
                       num_devices=8)
        gt_d = nc.dram_tensor("gt", [128, FDH], U16,
                              kind="ExternalInput").ap()
        net_d = nc.dram_tensor("net", [128, 4 * FDO], F32,
                               kind="ExternalInput").ap()
        wn_d = nc.dram_tensor("wn", [128, 384], BF16,
                              kind="ExternalInput").ap()
        wp_d = nc.dram_tensor("wp", [128, 512], BF16,
                              kind="ExternalInput").ap()
        out_d = nc.dram_tensor("out", [128, 4], F32,
                               kind="ExternalOutput").ap()
        x_d = nc.dram_tensor("xs", [128, 3 * FDO], U16,
                             kind="ExternalOutput").ap()
        with TileContext(nc) as tc:
            _body(tc, gt_d, net_d, wn_d, wp_d, out_d, x_d)
        nc.compile()
        _NC = nc
    return _NC


def _in_maps(net_output, gt):
    bf = ml_dtypes.bfloat16
    b0 = (np.eye(128) + LB * (np.eye(128, k=1) + np.eye(128, k=-1))
          + LB ** 4 * (np.eye(128, k=2) + np.eye(128, k=-2)))
    wn = 16.0 * np.concatenate([b0, LB * b0, LB ** 4 * b0],
                               axis=1).astype(np.float64)
    wn = wn.astype(bf)
    p0 = np.eye(128) + LB * (np.eye(128, k=1) + np.eye(128, k=-1))
    ey = np.eye(128)
    wp = np.concatenate([p0, LB * p0, ey, LB * ey], axis=1).astype(bf)

    gtu = np.asarray(gt)[:, 0].astype(np.uint16)
    gtz = np.pad(gtu, ((0, 0), (0, 0), (0, 0), (H, H)), constant_values=255)
    maps = []
    for core in range(8):
        b, zs = core // 4, core % 4
        z0 = zs * ZO
        sl = gtz[b, :, :, z0:z0 + ZT]                       # [128, 128, 28]
        gts = np.pad(sl, ((0, 0), (2, 2), (0, 0)), constant_values=255)
        nets = np.ascontiguousarray(
            np.transpose(net_output[b, :, :, :, z0:z0 + ZO], (1, 0, 2, 3)))
        maps.append({
            "gt": gts.reshape(128, FDH),
            "net": nets.reshape(128, 4 * FDO).astype(np.float32),
            "wn": wn, "wp": wp,
        })
    return maps


def _pos_window_ok(gtu):
    """True iff no foreground voxel (any class 1..3) has its entire 3^3
    neighborhood foreground-of-the-same-class (pos2 <= 3 everywhere,
    out-of-volume treated as foreground)."""
    for c in range(1, C):
        m = gtu == c
        p = np.pad(m, ((0, 0), (1, 1), (1, 1), (1, 1)), constant_values=True)
        ex = p[:, :-2] & p[:, 1:-1] & p[:, 2:]
        ey = ex[:, :, :-2] & ex[:, :, 1:-1] & ex[:, :, 2:]
        ez = ey[:, :, :, :-2] & ey[:, :, :, 1:-1] & ey[:, :, :, 2:]
        if (m & ez).any():
            return False
    return True


def _fallback(net_output, gt):
    """Exact host computation (safety net if windowed-EDT verification
    fails)."""
    from scipy import ndimage
    net = np.asarray(net_output, np.float64)
    g = np.asarray(gt)[:, 0]
    e = np.exp(net - net.max(axis=1, keepdims=True))
    probs = e / e.sum(axis=1, keepdims=True)
    tot = 0.0
    for b in range(B):
        for c in range(1, C):
            m = g[b] == c
            if not m.any():
                continue
            pos = ndimage.distance_transform_edt(m)
            neg = ndimage.distance_transform_edt(~m)
            er = ndimage.binary_erosion(
                m, structure=ndimage.generate_binary_structure(3, 1),
                border_value=1)
            phi = np.where(m & ~er, 0.0, neg - pos)
            tot += float((probs[b, c] * phi).sum())
    return np.float32(tot / NVOX)


def kernel(net_output, gt, _spmd_result=[None]):
    nc = _get_nc()
    res = bass_utils.run_bass_kernel_spmd(nc, _in_maps(net_output, gt),
                                          core_ids=list(range(8)))
    _spmd_result[0] = res
    total, ok = 0.0, True
    for r in res.results:
        o = np.asarray(r["out"], np.float64)
        total += o[:, 0:3].sum()
        xs = np.asarray(r["xs"])
        ok &= bool(xs.min() >= 8)       # x = 16 - m; need m <= 8 everywhere
    ok = ok and _pos_window_ok(np.asarray(gt)[:, 0])
    if not ok:
        return _fallback(net_output, gt)
    return np.float32(total / NVOX)


# revision 32
# speedup vs baseline: 1.1070x; 1.1070x over previous
"""Boundary-distance loss (BDLoss) on 8 Trainium2 NeuronCores.

Math (matches the reference):
  probs = softmax(net_output, axis=1)
  onehot_c = (gt == c)
  posdis = EDT(onehot_c), negdis = EDT(~onehot_c)
  phi = where(inner_boundary, 0, negdis - posdis), zeroed if class absent
  out  = mean(probs[:, 1:] * phi[:, 1:])

Algorithm (exponential-weight separable convolution):
  * NEG field: E = conv3d(fpos, beta^-d^2) over the 5^3 box with beta = 2^8.
    Since all weights are powers of two and counts per distance-shell are
    < 32, the EXPONENT FIELD of E encodes m = min d^2 exactly:
    x := (bits(E) + 512) >> 10 == 16 - m (x == 0 for "nothing in box").
    The conv is separable: the x-taps ride in banded 128x128 matmul weights,
    the y-taps are 5 shifted matmul passes accumulated in PSUM (all on PE),
    and the z-taps are 4 pair-adds + 2 scales on DVE.  sqrt(m) is a single
    ACT pass: sqrt(-1*x + 16).
  * POS field: posE = conv3d(fneg, beta^-d^2) over the 3^3 box (fneg = "bg
    for class c", with OOV/pads forced to 0 so out-of-volume never counts
    as background).  Then the boundary-zeroed positive distance is
    posd = sqrt2*[posE < 2^-9] + (sqrt3-sqrt2)*[posE < 2^-17]
    (thresholds = "no bg at d^2<=1 / <=2"), auto-zero on bg voxels because
    the center tap makes posE >= 1 there.
  * phi = sqrt(m) - posd; result = sum(probs_c * phi) via a DVE
    tensor_scalar with accum_out per class; host sums across cores.
  * Verification: x-fields are shipped; min(x) >= 8 iff the windowed box
    EDT is exact (neg^2 <= 8 everywhere); pos exactness (pos^2 <= 3) is
    checked with numpy erosion.  On failure -> exact scipy fallback.

Sharding: core = (b, z-slab): b = core//4, z0 = 24*(core%4).  gt is sent as
uint16 with a 2-plane z halo and 2-column y pad of value 255.
"""

import numpy as np
import ml_dtypes

import concourse.bacc as bacc
import concourse.mybir as mybir
from concourse.tile import TileContext
from concourse import bass_utils

F32 = mybir.dt.float32
BF16 = mybir.dt.bfloat16
U16 = mybir.dt.uint16
AL = mybir.AluOpType
AF = mybir.ActivationFunctionType

B, C, X, Y, Z = 2, 4, 128, 128, 96
ZO = 24            # output z-planes per core
H = 2              # z halo
ZT = ZO + 2 * H    # 28 z-planes on chip
YP = Y + 4         # y padded to 132
FDH = YP * ZT      # 3696 cols of the padded mask tile
FDO = Y * ZO       # 3072 cols of a dense output tile
NVOX = B * (C - 1) * X * Y * Z
SQ2 = float(np.sqrt(2.0))
SQ3 = float(np.sqrt(3.0))
LB = 2.0 ** -8     # conv base beta^-1
T1 = 2.0 ** -9     # pos threshold: no bg at d^2 <= 1
T2 = 2.0 ** -17    # pos threshold: no bg at d^2 <= 2
CW = 3584          # conv output cols: y in [2,130) x z in [0,28)
C_LO = 2 * ZT      # first output col
CHUNK = 1024       # psum chunk (2 banks)
SUB = 512          # matmul sub-block (1 bank)

NEG_PASSES = [(2, -2), (2, 2), (1, -1), (1, 1), (0, 0)]   # (wn band idx, dy)
POS_PASSES = [(1, -1), (1, 1), (0, 0)]                    # (wp band idx, dy)


def _body(tc, gt_d, net_d, wn_d, wp_d, out_d, x_d):
    nc = tc.nc
    with tc.tile_pool(name="main", bufs=1) as pool, \
         tc.tile_pool(name="rot", bufs=2) as rot, \
         tc.tile_pool(name="nps", bufs=2, space="PSUM") as nps, \
         tc.tile_pool(name="pps", bufs=2, space="PSUM") as pps:

        # ---- input loads --------------------------------------------
        gt16 = pool.tile([128, FDH], U16, tag="gt")
        for gg in range(2):
            sl = slice(gg * FDH // 2, (gg + 1) * FDH // 2)
            nc.sync.dma_start(gt16[:, sl], gt_d[:, sl])
        wn_t = pool.tile([128, 384], BF16, tag="wn")
        nc.sync.dma_start(wn_t[:, :], wn_d)
        wp_t = pool.tile([128, 512], BF16, tag="wp")
        nc.sync.dma_start(wp_t[:, :], wp_d)
        b16 = pool.tile([128, 1], F32, tag="b16")
        nc.vector.memset(b16[:, :], 16.0)
        junk = pool.tile([128, 384], BF16, tag="junk")
        nc.vector.memset(junk[:, :], 0.0)

        # ---- softmax exps: 2 now, 6 after class-1 copy emission ------
        e_t = pool.tile([128, 4 * FDO], BF16, tag="et")
        HF = FDO // 2

        def emit_exps(rng):
            for cc in rng:
                st = rot.tile([128, HF], F32, tag="stage", bufs=2)
                nc.sync.dma_start(st[:, :], net_d[:, cc * HF:(cc + 1) * HF])
                nc.scalar.activation(e_t[:, cc * HF:(cc + 1) * HF], st[:, :],
                                     AF.Exp)

        emit_exps(range(2))

        # ---- PE warm-up: keep the tensor engine busy from t~0.6us so
        # the p-state model ramps to full speed before the real convs
        # (a stalled wait resets the ramp; these chain into chunk 0's
        # psum, which the first real pass resets via start=True).
        warm = nps.tile([128, CHUNK], F32, tag="exyps", bufs=2)
        for _ in range(13):
            nc.tensor.matmul(warm[:, 0:384], junk[:, 0:128], junk[:, :],
                             start=True, stop=True)

        gtv = gt16[:, :].rearrange("p (y z) -> p y z", z=ZT)

        def build_masks(c, nch=1):
            fpos = rot.tile([128, FDH], BF16, tag="fpos", bufs=3)
            fneg = rot.tile([128, FDH], BF16, tag="fneg", bufs=3)
            for g in range(nch):
                sl = slice(g * FDH // nch, (g + 1) * FDH // nch)
                nc.vector.tensor_scalar(fpos[:, sl], gt16[:, sl], c, None,
                                        AL.is_equal)
            for g in range(nch):
                sl = slice(g * FDH // nch, (g + 1) * FDH // nch)
                nc.vector.tensor_scalar(fneg[:, sl], gt16[:, sl], c, None,
                                        AL.not_equal)
            # y pads -> 0 (OOV is never background)
            nc.gpsimd.memset(fneg[:, 0:2 * ZT], 0.0)
            nc.gpsimd.memset(fneg[:, 130 * ZT:132 * ZT], 0.0)
            return fpos, fneg

        def halo_fix(fpos, fneg):
            # z halo -> V - fpos (0 on OOV pad, unchanged on real data)
            fnv = fneg[:, :].rearrange("p (y z) -> p y z", z=ZT)
            fpv = fpos[:, :].rearrange("p (y z) -> p y z", z=ZT)
            nc.vector.tensor_tensor(fnv[:, :, 0:2], vhv[:, :, 0:2],
                                    fpv[:, :, 0:2], AL.subtract)
            nc.vector.tensor_tensor(fnv[:, :, ZT - 2:ZT], vhv[:, :, 2:4],
                                    fpv[:, :, ZT - 2:ZT], AL.subtract)

        den = pool.tile([128, FDO], BF16, tag="den")
        out_t = pool.tile([128, 4], F32, tag="out")
        scr = e_t[:, 0:FDO]        # e0 slot is dead once den exists
        xzs, gs = {}, {}
        masks = [build_masks(1, nch=2)]
        # valid-mask z-halo planes (for the fneg halo fix), after masks so
        # the DVE queue serves class-1's masks first
        vh = pool.tile([128, 4 * YP], BF16, tag="vh")
        vhv = vh[:, :].rearrange("p (y z) -> p y z", z=4)
        nc.vector.tensor_scalar(vhv[:, :, 0:2], gtv[:, :, 0:2], 3, None,
                                AL.is_le)
        nc.vector.tensor_scalar(vhv[:, :, 2:4], gtv[:, :, ZT - 2:ZT], 3,
                                None, AL.is_le)
        halo_fix(*masks[0])

        def conv_xy(mask, passes, wmat, tag, ppool):
            """x-band (x) y-shift (xy) conv passes -> SBUF bf16 [128, CW]."""
            sb = rot.tile([128, CW], BF16, tag=tag, bufs=2)
            off = 0
            while off < CW:
                w = min(CHUNK, CW - off)
                ps = ppool.tile([128, CHUNK], F32, tag=tag + "ps", bufs=2)
                for pi, (wi, dy) in enumerate(passes):
                    first, last = pi == 0, pi == len(passes) - 1
                    for mm in range(0, w, SUB):
                        mw = min(SUB, w - mm)
                        a = C_LO + dy * ZT + off + mm
                        nc.tensor.matmul(ps[:, mm:mm + mw],
                                         wmat[:, 128 * wi:128 * (wi + 1)],
                                         mask[:, a:a + mw],
                                         start=first, stop=last)
                nc.scalar.activation(sb[:, off:off + w], ps[:, 0:w], AF.Copy)
                off += w
            return sb

        YH = Y // 2

        def zconv5(sb):
            """5-tap z-conv on DVE (y-halves) -> flat [128, FDO] bf16."""
            v = sb[:, :].rearrange("p (y z) -> p y z", z=ZT)
            a = rot.tile([128, FDO], BF16, tag="A", bufs=2)
            b = rot.tile([128, FDO], BF16, tag="Bt", bufs=2)
            av = a[:, :].rearrange("p (y z) -> p y z", z=ZO)
            bv = b[:, :].rearrange("p (y z) -> p y z", z=ZO)
            for y0 in (0, YH):
                ys = slice(y0, y0 + YH)
                nc.vector.tensor_tensor(av[:, ys, :], v[:, ys, 1:1 + ZO],
                                        v[:, ys, 3:3 + ZO], AL.add)
                nc.vector.tensor_tensor(bv[:, ys, :], v[:, ys, 0:ZO],
                                        v[:, ys, 4:4 + ZO], AL.add)
                nc.vector.tensor_scalar(av[:, ys, :], av[:, ys, :], LB,
                                        None, AL.mult)
                nc.vector.tensor_scalar(bv[:, ys, :], bv[:, ys, :], LB ** 4,
                                        None, AL.mult)
                nc.vector.tensor_tensor(av[:, ys, :], v[:, ys, 2:2 + ZO],
                                        av[:, ys, :], AL.add)
                nc.vector.tensor_tensor(av[:, ys, :], av[:, ys, :],
                                        bv[:, ys, :], AL.add)
            return a, b

        def zconv3(sb):
            """3-tap z-conv on DVE (y-halves) -> flat [128, FDO] bf16."""
            v = sb[:, :].rearrange("p (y z) -> p y z", z=ZT)
            a = rot.tile([128, FDO], BF16, tag="Bt", bufs=2)
            av = a[:, :].rearrange("p (y z) -> p y z", z=ZO)
            for y0 in (0, YH):
                ys = slice(y0, y0 + YH)
                nc.vector.tensor_tensor(av[:, ys, :], v[:, ys, 1:1 + ZO],
                                        v[:, ys, 3:3 + ZO], AL.add)
                nc.vector.tensor_scalar(av[:, ys, :], av[:, ys, :], LB,
                                        None, AL.mult)
                nc.vector.tensor_tensor(av[:, ys, :], v[:, ys, 2:2 + ZO],
                                        av[:, ys, :], AL.add)
            return a

        for ci in range(3):
            fpos, fneg = masks[ci]
            exy = conv_xy(fpos, NEG_PASSES, wn_t, "exy", nps)
            cxy = conv_xy(fneg, POS_PASSES, wp_t, "cxy", pps)
            if ci < 2:
                masks.append(build_masks(ci + 2))
                halo_fix(*masks[ci + 1])
            if ci == 0:
                emit_exps(range(2, 5))
            if ci == 1:
                emit_exps(range(5, 8))
                # den + 1/den + probability folds (Pool + ACT, off-path)
                nc.gpsimd.tensor_tensor(den[:, :], e_t[:, 0:FDO],
                                        e_t[:, FDO:2 * FDO], AL.add)
                nc.gpsimd.tensor_tensor(den[:, :], den[:, :],
                                        e_t[:, 2 * FDO:3 * FDO], AL.add)
                nc.gpsimd.tensor_tensor(den[:, :], den[:, :],
                                        e_t[:, 3 * FDO:4 * FDO], AL.add)
                for hh in range(2):
                    sl = slice(hh * HF, (hh + 1) * HF)
                    lh = rot.tile([128, HF], F32, tag="stage", bufs=2)
                    nc.scalar.activation(lh[:, :], den[:, sl], AF.Ln)
                    nc.scalar.activation(den[:, sl], lh[:, :], AF.Exp,
                                         scale=-1.0)
                for c in (1, 2, 3):
                    esl = e_t[:, c * FDO:(c + 1) * FDO]
                    nc.gpsimd.tensor_tensor(esl, esl, den[:, :], AL.mult)
            # neg z + exponent decode -> x = 16 - m (u16)
            ez, bt = zconv5(exy)
            xz = rot.tile([128, FDO], U16, tag="xz", bufs=3)
            nc.vector.tensor_scalar(xz[:, :], ez[:, :].bitcast(U16), 10,
                                    None, AL.logical_shift_right)
            nc.sync.dma_start(x_d[:, ci * FDO:(ci + 1) * FDO], xz[:, :])
            xzs[ci] = xz
            # pos z + thresholds -> g = posd'
            pe = zconv3(cxy)
            g1 = rot.tile([128, FDO], BF16, tag="g", bufs=3)
            g2 = rot.tile([128, FDO], BF16, tag="Bt", bufs=2)
            nc.vector.tensor_scalar(g1[:, :], pe[:, :], T1, SQ2, AL.is_lt,
                                    AL.mult)
            nc.vector.tensor_scalar(g2[:, :], pe[:, :], T2, SQ3 - SQ2,
                                    AL.is_lt, AL.mult)
            nc.vector.tensor_tensor(g1[:, :], g1[:, :], g2[:, :], AL.add)
            gs[ci] = g1

        # ---- sqrts batched (one act-table switch), then tails --------
        for ci in range(3):
            sn = rot.tile([128, FDO], BF16, tag="sn", bufs=2)
            nc.scalar.activation(sn[:, :], xzs[ci][:, :], AF.Sqrt,
                                 bias=b16[:, 0:1], scale=-1.0)
            g = gs[ci]
            nc.vector.tensor_tensor(g[:, :], sn[:, :], g[:, :],
                                    AL.subtract)
            nc.vector.tensor_tensor(g[:, :], g[:, :],
                                    e_t[:, (ci + 1) * FDO:(ci + 2) * FDO],
                                    AL.mult)
            nc.vector.tensor_scalar(scr[:, :], g[:, :], 1.0, 0.0, AL.mult,
                                    AL.add, accum_out=out_t[:, ci:ci + 1])
        nc.vector.memset(out_t[:, 3:4], 0.0)
        nc.sync.dma_start(out_d[:, :], out_t[:, :])


_NC = None


def _get_nc():
    global _NC
    if _NC is None:
        nc = bacc.Bacc("TRN2", target_bir_lowering=False, debug=False,
                       num_devices=8)
        gt_d = nc.dram_tensor("gt", [128, FDH], U16,
                              kind="ExternalInput").ap()
        net_d = nc.dram_tensor("net", [128, 4 * FDO], F32,
                               kind="ExternalInput").ap()
        wn_d = nc.dram_tensor("wn", [128, 384], BF16,
                              kind="ExternalInput").ap()
        wp_d = nc.dram_tensor("wp", [128, 512], BF16,
                              kind="ExternalInput").ap()
        out_d = nc.dram_tensor("out", [128, 4], F32,
                               kind="ExternalOutput").ap()
        x_d = nc.dram_tensor("xs", [128, 3 * FDO], U16,
                             kind="ExternalOutput").ap()
        with TileContext(nc) as tc:
            _body(tc, gt_d, net_d, wn_d, wp_d, out_d, x_d)
        nc.compile()
        _NC = nc
    return _NC


def _in_maps(net_output, gt):
    bf = ml_dtypes.bfloat16
    b0 = (np.eye(128) + LB * (np.eye(128, k=1) + np.eye(128, k=-1))
          + LB ** 4 * (np.eye(128, k=2) + np.eye(128, k=-2)))
    wn = 16.0 * np.concatenate([b0, LB * b0, LB ** 4 * b0],
                               axis=1).astype(np.float64)
    wn = wn.astype(bf)
    p0 = np.eye(128) + LB * (np.eye(128, k=1) + np.eye(128, k=-1))
    ey = np.eye(128)
    wp = np.concatenate([p0, LB * p0, ey, LB * ey], axis=1).astype(bf)

    gtu = np.asarray(gt)[:, 0].astype(np.uint16)
    gtz = np.pad(gtu, ((0, 0), (0, 0), (0, 0), (H, H)), constant_values=255)
    maps = []
    for core in range(8):
        b, zs = core // 4, core % 4
        z0 = zs * ZO
        sl = gtz[b, :, :, z0:z0 + ZT]                       # [128, 128, 28]
        gts = np.pad(sl, ((0, 0), (2, 2), (0, 0)), constant_values=255)
        nets = np.ascontiguousarray(
            np.transpose(net_output[b, :, :, :, z0:z0 + ZO], (1, 0, 2, 3)))
        maps.append({
            "gt": gts.reshape(128, FDH),
            "net": nets.reshape(128, 4 * FDO).astype(np.float32),
            "wn": wn, "wp": wp,
        })
    return maps


def _pos_window_ok(gtu):
    """True iff no foreground voxel (any class 1..3) has its entire 3^3
    neighborhood foreground-of-the-same-class (pos2 <= 3 everywhere,
    out-of-volume treated as foreground)."""
    for c in range(1, C):
        m = gtu == c
        p = np.pad(m, ((0, 0), (1, 1), (1, 1), (1, 1)), constant_values=True)
        ex = p[:, :-2] & p[:, 1:-1] & p[:, 2:]
        ey = ex[:, :, :-2] & ex[:, :, 1:-1] & ex[:, :, 2:]
        ez = ey[:, :, :, :-2] & ey[:, :, :, 1:-1] & ey[:, :, :, 2:]
        if (m & ez).any():
            return False
    return True


def _fallback(net_output, gt):
    """Exact host computation (safety net if windowed-EDT verification
    fails)."""
    from scipy import ndimage
    net = np.asarray(net_output, np.float64)
    g = np.asarray(gt)[:, 0]
    e = np.exp(net - net.max(axis=1, keepdims=True))
    probs = e / e.sum(axis=1, keepdims=True)
    tot = 0.0
    for b in range(B):
        for c in range(1, C):
            m = g[b] == c
            if not m.any():
                continue
            pos = ndimage.distance_transform_edt(m)
            neg = ndimage.distance_transform_edt(~m)
            er = ndimage.binary_erosion(
                m, structure=ndimage.generate_binary_structure(3, 1),
                border_value=1)
            phi = np.where(m & ~er, 0.0, neg - pos)
            tot += float((probs[b, c] * phi).sum())
    return np.float32(tot / NVOX)


def kernel(net_output, gt, _spmd_result=[None]):
    nc = _get_nc()
    res = bass_utils.run_bass_kernel_spmd(nc, _in_maps(net_output, gt),
                                          core_ids=list(range(8)))
    _spmd_result[0] = res
    total, ok = 0.0, True
    for r in res.results:
        o = np.asarray(r["out"], np.float64)
        total += o[:, 0:3].sum()
        xs = np.asarray(r["xs"])
        ok &= bool(xs.min() >= 8)       # x = 16 - m; need m <= 8 everywhere
    ok = ok and _pos_window_ok(np.asarray(gt)[:, 0])
    if not ok:
        return _fallback(net_output, gt)
    return np.float32(total / NVOX)


# revision 33
# speedup vs baseline: 1.1179x; 1.0099x over previous
"""Boundary-distance loss (BDLoss) on 8 Trainium2 NeuronCores.

Math (matches the reference):
  probs = softmax(net_output, axis=1)
  onehot_c = (gt == c)
  posdis = EDT(onehot_c), negdis = EDT(~onehot_c)
  phi = where(inner_boundary, 0, negdis - posdis), zeroed if class absent
  out  = mean(probs[:, 1:] * phi[:, 1:])

Algorithm (exponential-weight separable convolution):
  * NEG field: E = conv3d(fpos, beta^-d^2) over the 5^3 box with beta = 2^8.
    Since all weights are powers of two and counts per distance-shell are
    < 32, the EXPONENT FIELD of E encodes m = min d^2 exactly:
    x := (bits(E) + 512) >> 10 == 16 - m (x == 0 for "nothing in box").
    The conv is separable: the x-taps ride in banded 128x128 matmul weights,
    the y-taps are 5 shifted matmul passes accumulated in PSUM (all on PE),
    and the z-taps are 4 pair-adds + 2 scales on DVE.  sqrt(m) is a single
    ACT pass: sqrt(-1*x + 16).
  * POS field: posE = conv3d(fneg, beta^-d^2) over the 3^3 box (fneg = "bg
    for class c", with OOV/pads forced to 0 so out-of-volume never counts
    as background).  Then the boundary-zeroed positive distance is
    posd = sqrt2*[posE < 2^-9] + (sqrt3-sqrt2)*[posE < 2^-17]
    (thresholds = "no bg at d^2<=1 / <=2"), auto-zero on bg voxels because
    the center tap makes posE >= 1 there.
  * phi = sqrt(m) - posd; result = sum(probs_c * phi) via a DVE
    tensor_scalar with accum_out per class; host sums across cores.
  * Verification: x-fields are shipped; min(x) >= 8 iff the windowed box
    EDT is exact (neg^2 <= 8 everywhere); pos exactness (pos^2 <= 3) is
    checked with numpy erosion.  On failure -> exact scipy fallback.

Sharding: core = (b, z-slab): b = core//4, z0 = 24*(core%4).  gt is sent as
uint16 with a 2-plane z halo and 2-column y pad of value 255.
"""

import numpy as np
import ml_dtypes

import concourse.bacc as bacc
import concourse.mybir as mybir
from concourse.tile import TileContext
from concourse import bass_utils

F32 = mybir.dt.float32
BF16 = mybir.dt.bfloat16
U16 = mybir.dt.uint16
AL = mybir.AluOpType
AF = mybir.ActivationFunctionType

B, C, X, Y, Z = 2, 4, 128, 128, 96
ZO = 24            # output z-planes per core
H = 2              # z halo
ZT = ZO + 2 * H    # 28 z-planes on chip
YP = Y + 4         # y padded to 132
FDH = YP * ZT      # 3696 cols of the padded mask tile
FDO = Y * ZO       # 3072 cols of a dense output tile
NVOX = B * (C - 1) * X * Y * Z
SQ2 = float(np.sqrt(2.0))
SQ3 = float(np.sqrt(3.0))
LB = 2.0 ** -8     # conv base beta^-1
T1 = 2.0 ** -9     # pos threshold: no bg at d^2 <= 1
T2 = 2.0 ** -17    # pos threshold: no bg at d^2 <= 2
CW = 3584          # conv output cols: y in [2,130) x z in [0,28)
C_LO = 2 * ZT      # first output col
CHUNK = 1024       # psum chunk (2 banks)
SUB = 512          # matmul sub-block (1 bank)

NEG_PASSES = [(2, -2), (2, 2), (1, -1), (1, 1), (0, 0)]   # (wn band idx, dy)
POS_PASSES = [(1, -1), (1, 1), (0, 0)]                    # (wp band idx, dy)


def _body(tc, gt_d, net_d, wn_d, wp_d, out_d, x_d):
    nc = tc.nc
    with tc.tile_pool(name="main", bufs=1) as pool, \
         tc.tile_pool(name="rot", bufs=2) as rot, \
         tc.tile_pool(name="nps", bufs=2, space="PSUM") as nps, \
         tc.tile_pool(name="pps", bufs=2, space="PSUM") as pps:

        # ---- input loads --------------------------------------------
        gt16 = pool.tile([128, FDH], U16, tag="gt")
        for gg in range(2):
            sl = slice(gg * FDH // 2, (gg + 1) * FDH // 2)
            nc.sync.dma_start(gt16[:, sl], gt_d[:, sl])
        wn_t = pool.tile([128, 384], BF16, tag="wn")
        nc.sync.dma_start(wn_t[:, :], wn_d)
        wp_t = pool.tile([128, 512], BF16, tag="wp")
        nc.sync.dma_start(wp_t[:, :], wp_d)
        b16 = pool.tile([128, 1], F32, tag="b16")
        nc.vector.memset(b16[:, :], 16.0)
        junk = pool.tile([128, 384], BF16, tag="junk")
        nc.vector.memset(junk[:, :], 0.0)

        # ---- softmax exps: 2 now, 6 after class-1 copy emission ------
        e_t = pool.tile([128, 4 * FDO], BF16, tag="et")
        HF = FDO // 2

        def emit_exps(rng):
            for cc in rng:
                st = rot.tile([128, HF], F32, tag="stage", bufs=2)
                nc.sync.dma_start(st[:, :], net_d[:, cc * HF:(cc + 1) * HF])
                nc.scalar.activation(e_t[:, cc * HF:(cc + 1) * HF], st[:, :],
                                     AF.Exp)

        emit_exps(range(2))

        # ---- PE warm-up: keep the tensor engine busy from t~0.6us so
        # the p-state model ramps to full speed before the real convs
        # (a stalled wait resets the ramp; these chain into chunk 0's
        # psum, which the first real pass resets via start=True).
        warm = nps.tile([128, CHUNK], F32, tag="exyps", bufs=2)
        for _ in range(13):
            nc.tensor.matmul(warm[:, 0:384], junk[:, 0:128], junk[:, :],
                             start=True, stop=True)

        gtv = gt16[:, :].rearrange("p (y z) -> p y z", z=ZT)

        def build_masks(c, nch=1):
            fpos = rot.tile([128, FDH], BF16, tag="fpos", bufs=3)
            fneg = rot.tile([128, FDH], BF16, tag="fneg", bufs=3)
            for g in range(nch):
                sl = slice(g * FDH // nch, (g + 1) * FDH // nch)
                nc.vector.tensor_scalar(fpos[:, sl], gt16[:, sl], c, None,
                                        AL.is_equal)
            for g in range(nch):
                sl = slice(g * FDH // nch, (g + 1) * FDH // nch)
                nc.vector.tensor_scalar(fneg[:, sl], gt16[:, sl], c, None,
                                        AL.not_equal)
            # y pads -> 0 (OOV is never background)
            nc.gpsimd.memset(fneg[:, 0:2 * ZT], 0.0)
            nc.gpsimd.memset(fneg[:, 130 * ZT:132 * ZT], 0.0)
            return fpos, fneg

        def halo_fix(fpos, fneg):
            # z halo -> V - fpos (0 on OOV pad, unchanged on real data)
            fnv = fneg[:, :].rearrange("p (y z) -> p y z", z=ZT)
            fpv = fpos[:, :].rearrange("p (y z) -> p y z", z=ZT)
            nc.vector.tensor_tensor(fnv[:, :, 0:2], vhv[:, :, 0:2],
                                    fpv[:, :, 0:2], AL.subtract)
            nc.vector.tensor_tensor(fnv[:, :, ZT - 2:ZT], vhv[:, :, 2:4],
                                    fpv[:, :, ZT - 2:ZT], AL.subtract)

        den = pool.tile([128, FDO], BF16, tag="den")
        out_t = pool.tile([128, 4], F32, tag="out")
        scr = e_t[:, 0:FDO]        # e0 slot is dead once den exists
        xzs, gs = {}, {}
        masks = [build_masks(1, nch=2)]
        # valid-mask z-halo planes (for the fneg halo fix), after masks so
        # the DVE queue serves class-1's masks first
        vh = pool.tile([128, 4 * YP], BF16, tag="vh")
        vhv = vh[:, :].rearrange("p (y z) -> p y z", z=4)
        nc.vector.tensor_scalar(vhv[:, :, 0:2], gtv[:, :, 0:2], 3, None,
                                AL.is_le)
        nc.vector.tensor_scalar(vhv[:, :, 2:4], gtv[:, :, ZT - 2:ZT], 3,
                                None, AL.is_le)
        halo_fix(*masks[0])

        def conv_xy(mask, passes, wmat, tag, ppool):
            """x-band (x) y-shift (xy) conv passes -> SBUF bf16 [128, CW]."""
            sb = rot.tile([128, CW], BF16, tag=tag, bufs=2)
            off = 0
            while off < CW:
                w = min(CHUNK, CW - off)
                ps = ppool.tile([128, CHUNK], F32, tag=tag + "ps", bufs=2)
                for pi, (wi, dy) in enumerate(passes):
                    first, last = pi == 0, pi == len(passes) - 1
                    for mm in range(0, w, SUB):
                        mw = min(SUB, w - mm)
                        a = C_LO + dy * ZT + off + mm
                        nc.tensor.matmul(ps[:, mm:mm + mw],
                                         wmat[:, 128 * wi:128 * (wi + 1)],
                                         mask[:, a:a + mw],
                                         start=first, stop=last)
                nc.scalar.activation(sb[:, off:off + w], ps[:, 0:w], AF.Copy)
                off += w
            return sb

        YH = Y // 2

        def zconv5(sb, nh=1):
            """5-tap z-conv on DVE (y-split for early start) -> [128,FDO]."""
            v = sb[:, :].rearrange("p (y z) -> p y z", z=ZT)
            a = rot.tile([128, FDO], BF16, tag="A", bufs=2)
            b = rot.tile([128, FDO], BF16, tag="Bt", bufs=2)
            av = a[:, :].rearrange("p (y z) -> p y z", z=ZO)
            bv = b[:, :].rearrange("p (y z) -> p y z", z=ZO)
            for y0 in range(0, Y, Y // nh):
                ys = slice(y0, y0 + Y // nh)
                nc.vector.tensor_tensor(av[:, ys, :], v[:, ys, 1:1 + ZO],
                                        v[:, ys, 3:3 + ZO], AL.add)
                nc.vector.tensor_tensor(bv[:, ys, :], v[:, ys, 0:ZO],
                                        v[:, ys, 4:4 + ZO], AL.add)
                nc.vector.tensor_scalar(av[:, ys, :], av[:, ys, :], LB,
                                        None, AL.mult)
                nc.vector.tensor_scalar(bv[:, ys, :], bv[:, ys, :], LB ** 4,
                                        None, AL.mult)
                nc.vector.tensor_tensor(av[:, ys, :], v[:, ys, 2:2 + ZO],
                                        av[:, ys, :], AL.add)
                nc.vector.tensor_tensor(av[:, ys, :], av[:, ys, :],
                                        bv[:, ys, :], AL.add)
            return a, b

        def zconv3(sb, nh=1):
            """3-tap z-conv on DVE -> flat [128, FDO] bf16."""
            v = sb[:, :].rearrange("p (y z) -> p y z", z=ZT)
            a = rot.tile([128, FDO], BF16, tag="Bt", bufs=2)
            av = a[:, :].rearrange("p (y z) -> p y z", z=ZO)
            for y0 in range(0, Y, Y // nh):
                ys = slice(y0, y0 + Y // nh)
                nc.vector.tensor_tensor(av[:, ys, :], v[:, ys, 1:1 + ZO],
                                        v[:, ys, 3:3 + ZO], AL.add)
                nc.vector.tensor_scalar(av[:, ys, :], av[:, ys, :], LB,
                                        None, AL.mult)
                nc.vector.tensor_tensor(av[:, ys, :], v[:, ys, 2:2 + ZO],
                                        av[:, ys, :], AL.add)
            return a

        for ci in range(3):
            fpos, fneg = masks[ci]
            exy = conv_xy(fpos, NEG_PASSES, wn_t, "exy", nps)
            cxy = conv_xy(fneg, POS_PASSES, wp_t, "cxy", pps)
            if ci < 2:
                masks.append(build_masks(ci + 2))
                halo_fix(*masks[ci + 1])
            if ci == 0:
                emit_exps(range(2, 5))
            if ci == 1:
                emit_exps(range(5, 8))
                # den + 1/den + probability folds (Pool + ACT, off-path)
                nc.gpsimd.tensor_tensor(den[:, :], e_t[:, 0:FDO],
                                        e_t[:, FDO:2 * FDO], AL.add)
                nc.gpsimd.tensor_tensor(den[:, :], den[:, :],
                                        e_t[:, 2 * FDO:3 * FDO], AL.add)
                nc.gpsimd.tensor_tensor(den[:, :], den[:, :],
                                        e_t[:, 3 * FDO:4 * FDO], AL.add)
                for hh in range(2):
                    sl = slice(hh * HF, (hh + 1) * HF)
                    lh = rot.tile([128, HF], F32, tag="stage", bufs=2)
                    nc.scalar.activation(lh[:, :], den[:, sl], AF.Ln)
                    nc.scalar.activation(den[:, sl], lh[:, :], AF.Exp,
                                         scale=-1.0)
                for c in (1, 2, 3):
                    esl = e_t[:, c * FDO:(c + 1) * FDO]
                    nc.gpsimd.tensor_tensor(esl, esl, den[:, :], AL.mult)
            # neg z + exponent decode -> x = 16 - m (u16)
            ez, bt = zconv5(exy, nh=2 if ci == 0 else 1)
            xz = rot.tile([128, FDO], U16, tag="xz", bufs=3)
            nc.vector.tensor_scalar(xz[:, :], ez[:, :].bitcast(U16), 10,
                                    None, AL.logical_shift_right)
            nc.sync.dma_start(x_d[:, ci * FDO:(ci + 1) * FDO], xz[:, :])
            xzs[ci] = xz
            # pos z + thresholds -> g = posd'
            pe = zconv3(cxy, nh=2 if ci == 0 else 1)
            g1 = rot.tile([128, FDO], BF16, tag="g", bufs=3)
            g2 = rot.tile([128, FDO], BF16, tag="Bt", bufs=2)
            nc.vector.tensor_scalar(g1[:, :], pe[:, :], T1, SQ2, AL.is_lt,
                                    AL.mult)
            nc.vector.tensor_scalar(g2[:, :], pe[:, :], T2, SQ3 - SQ2,
                                    AL.is_lt, AL.mult)
            nc.vector.tensor_tensor(g1[:, :], g1[:, :], g2[:, :], AL.add)
            gs[ci] = g1

        # ---- sqrts batched (one act-table switch), then tails --------
        for ci in range(3):
            sn = rot.tile([128, FDO], BF16, tag="sn", bufs=2)
            nc.scalar.activation(sn[:, :], xzs[ci][:, :], AF.Sqrt,
                                 bias=b16[:, 0:1], scale=-1.0)
            g = gs[ci]
            nc.vector.tensor_tensor(g[:, :], sn[:, :], g[:, :],
                                    AL.subtract)
            nc.vector.tensor_tensor(g[:, :], g[:, :],
                                    e_t[:, (ci + 1) * FDO:(ci + 2) * FDO],
                                    AL.mult)
            nc.vector.tensor_scalar(scr[:, :], g[:, :], 1.0, 0.0, AL.mult,
                                    AL.add, accum_out=out_t[:, ci:ci + 1])
        nc.vector.memset(out_t[:, 3:4], 0.0)
        nc.sync.dma_start(out_d[:, :], out_t[:, :])


_NC = None


def _get_nc():
    global _NC
    if _NC is None:
        nc = bacc.Bacc("TRN2", target_bir_lowering=False, debug=False,
                       num_devices=8)
        gt_d = nc.dram_tensor("gt", [128, FDH], U16,
                              kind="ExternalInput").ap()
        net_d = nc.dram_tensor("net", [128, 4 * FDO], F32,
                               kind="ExternalInput").ap()
        wn_d = nc.dram_tensor("wn", [128, 384], BF16,
                              kind="ExternalInput").ap()
        wp_d = nc.dram_tensor("wp", [128, 512], BF16,
                              kind="ExternalInput").ap()
        out_d = nc.dram_tensor("out", [128, 4], F32,
                               kind="ExternalOutput").ap()
        x_d = nc.dram_tensor("xs", [128, 3 * FDO], U16,
                             kind="ExternalOutput").ap()
        with TileContext(nc) as tc:
            _body(tc, gt_d, net_d, wn_d, wp_d, out_d, x_d)
        nc.compile()
        _NC = nc
    return _NC


def _in_maps(net_output, gt):
    bf = ml_dtypes.bfloat16
    b0 = (np.eye(128) + LB * (np.eye(128, k=1) + np.eye(128, k=-1))
          + LB ** 4 * (np.eye(128, k=2) + np.eye(128, k=-2)))
    wn = 16.0 * np.concatenate([b0, LB * b0, LB ** 4 * b0],
                               axis=1).astype(np.float64)
    wn = wn.astype(bf)
    p0 = np.eye(128) + LB * (np.eye(128, k=1) + np.eye(128, k=-1))
    ey = np.eye(128)
    wp = np.concatenate([p0, LB * p0, ey, LB * ey], axis=1).astype(bf)

    gtu = np.asarray(gt)[:, 0].astype(np.uint16)
    gtz = np.pad(gtu, ((0, 0), (0, 0), (0, 0), (H, H)), constant_values=255)
    maps = []
    for core in range(8):
        b, zs = core // 4, core % 4
        z0 = zs * ZO
        sl = gtz[b, :, :, z0:z0 + ZT]                       # [128, 128, 28]
        gts = np.pad(sl, ((0, 0), (2, 2), (0, 0)), constant_values=255)
        nets = np.ascontiguousarray(
            np.transpose(net_output[b, :, :, :, z0:z0 + ZO], (1, 0, 2, 3)))
        maps.append({
            "gt": gts.reshape(128, FDH),
            "net": nets.reshape(128, 4 * FDO).astype(np.float32),
            "wn": wn, "wp": wp,
        })
    return maps


def _pos_window_ok(gtu):
    """True iff no foreground voxel (any class 1..3) has its entire 3^3
    neighborhood foreground-of-the-same-class (pos2 <= 3 everywhere,
    out-of-volume treated as foreground)."""
    for c in range(1, C):
        m = gtu == c
        p = np.pad(m, ((0, 0), (1, 1), (1, 1), (1, 1)), constant_values=True)
        ex = p[:, :-2] & p[:, 1:-1] & p[:, 2:]
        ey = ex[:, :, :-2] & ex[:, :, 1:-1] & ex[:, :, 2:]
        ez = ey[:, :, :, :-2] & ey[:, :, :, 1:-1] & ey[:, :, :, 2:]
        if (m & ez).any():
            return False
    return True


def _fallback(net_output, gt):
    """Exact host computation (safety net if windowed-EDT verification
    fails)."""
    from scipy import ndimage
    net = np.asarray(net_output, np.float64)
    g = np.asarray(gt)[:, 0]
    e = np.exp(net - net.max(axis=1, keepdims=True))
    probs = e / e.sum(axis=1, keepdims=True)
    tot = 0.0
    for b in range(B):
        for c in range(1, C):
            m = g[b] == c
            if not m.any():
                continue
            pos = ndimage.distance_transform_edt(m)
            neg = ndimage.distance_transform_edt(~m)
            er = ndimage.binary_erosion(
                m, structure=ndimage.generate_binary_structure(3, 1),
                border_value=1)
            phi = np.where(m & ~er, 0.0, neg - pos)
            tot += float((probs[b, c] * phi).sum())
    return np.float32(tot / NVOX)


def kernel(net_output, gt, _spmd_result=[None]):
    nc = _get_nc()
    res = bass_utils.run_bass_kernel_spmd(nc, _in_maps(net_output, gt),
                                          core_ids=list(range(8)))
    _spmd_result[0] = res
    total, ok = 0.0, True
    for r in res.results:
        o = np.asarray(r["out"], np.float64)
        total += o[:, 0:3].sum()
        xs = np.asarray(r["xs"])
        ok &= bool(xs.min() >= 8)       # x = 16 - m; need m <= 8 everywhere
    ok = ok and _pos_window_ok(np.asarray(gt)[:, 0])
    if not ok:
        return _fallback(net_output, gt)
    return np.float32(total / NVOX)
